# revision 6
# baseline (speedup 1.0000x reference)
"""TRN2 Bass kernel for nn_LowRankStateMixing.

Per-core (data-parallel over batch, B=8 -> 8 cores, each core handles one
batch element, T=4096, H=D=1024):

  phase 0: LN stats (mean, rsig) for all 32 token tiles (Sqrt ACT table)
  phase 1 per 128-token tile (sigmoid/tanh ACT table):
    - LN apply -> xn (f16)
    - PE-transpose xn -> xnT  [h,t]
    - fp16 matmuls xnT.T @ Wcat -> u, v, g, t, r  (f32 psum)
    - delta_act = u*v (f16), delta_gh = f16(sigmoid(g)*tanh(t))
    - blocked cumsum via triangular-matmul + ones-outer-product carry
      (carry master kept in f32 sbuf, f16-rounded copy used as MM operand)
    - c = rt * (active + ghost)  (f16)
    - PE-transpose c -> cT [d,t]; fp16 matmul cT.T @ WoT -> out tile
"""
import numpy as np
from contextlib import ExitStack

B, T, H = 8, 4096, 1024
P = 128
NT = T // P           # 32 token tiles
NH = H // P           # 8 h-tiles
OC = 4097             # u + v(1024) + g(1024) + t(1024) + r(1024)
N_CORES = 8

_CACHE = {}


def _build():
    import concourse.mybir as mybir
    import concourse.tile as tile
    from concourse import bacc
    from concourse.masks import make_upper_triangular, make_identity

    dt = mybir.dt
    AF = mybir.ActivationFunctionType

    nc = bacc.Bacc("TRN2", target_bir_lowering=False, debug=False)
    x = nc.dram_tensor("x", [T, H], dt.float32, kind="ExternalInput").ap()
    wcat = nc.dram_tensor("wcat", [H, OC], dt.float16, kind="ExternalInput").ap()
    wot = nc.dram_tensor("wot", [H, H], dt.float16, kind="ExternalInput").ap()
    out = nc.dram_tensor("out", [T, H], dt.float32, kind="ExternalOutput").ap()
    alast = nc.dram_tensor("alast", [1, H], dt.float32, kind="ExternalOutput").ap()
    glast = nc.dram_tensor("glast", [1, H], dt.float16, kind="ExternalOutput").ap()

    with tile.TileContext(nc) as tc, ExitStack() as ctx:
        consts = ctx.enter_context(tc.tile_pool(name="consts", bufs=1))
        wpool = ctx.enter_context(tc.tile_pool(name="w", bufs=1))
        xpool = ctx.enter_context(tc.tile_pool(name="xp", bufs=3))
        stat = ctx.enter_context(tc.tile_pool(name="stat", bufs=4))
        work = ctx.enter_context(tc.tile_pool(name="work", bufs=2))
        carry = ctx.enter_context(tc.tile_pool(name="carry", bufs=1))
        pp = ctx.enter_context(tc.tile_pool(name="pp", bufs=2, space="PSUM"))
        pt = ctx.enter_context(tc.tile_pool(name="pt", bufs=1, space="PSUM"))
        pu = ctx.enter_context(tc.tile_pool(name="pu", bufs=1, space="PSUM"))
        pcs = ctx.enter_context(tc.tile_pool(name="pcs", bufs=2, space="PSUM"))
        pwo = ctx.enter_context(tc.tile_pool(name="pwo", bufs=1, space="PSUM"))
        ptot = ctx.enter_context(tc.tile_pool(name="ptot", bufs=1, space="PSUM"))

        # ---- constants ----
        u32 = consts.tile([P, P], dt.float32)
        make_upper_triangular(nc, u32[:], val=1.0, diag=True)
        umask = consts.tile([P, P], dt.float16)
        nc.vector.tensor_copy(umask[:], u32[:])
        id32 = consts.tile([P, P], dt.float32)
        make_identity(nc, id32[:])
        ident = consts.tile([P, P], dt.float16)
        nc.vector.tensor_copy(ident[:], id32[:])
        ones_col = consts.tile([1, P], dt.float16)
        nc.vector.memset(ones_col[:], 1.0)
        ones128 = consts.tile([P, 1], dt.float16)
        nc.vector.memset(ones128[:], 1.0)
        eps = consts.tile([P, 1], dt.float32)
        nc.vector.memset(eps[:], 1e-5)

        # ---- weights resident ----
        w_sb = wpool.tile([P, NH, OC], dt.float16)
        nc.sync.dma_start(w_sb[:], wcat.rearrange("(a p) o -> p a o", p=P))
        wo_sb = wpool.tile([P, NH, H], dt.float16)
        nc.sync.dma_start(wo_sb[:], wot.rearrange("(a p) o -> p a o", p=P))

        # ---- carries (f32 master + f16 rounded MM operand) ----
        ca = carry.tile([1, H], dt.float32)
        cg = carry.tile([1, H], dt.float32)
        cah = carry.tile([1, H], dt.float16)
        cgh = carry.tile([1, H], dt.float16)
        for t_ in (ca, cg, cah, cgh):
            nc.vector.memset(t_[:], 0.0)

        # ---- phase 0: LN stats for all tiles ----
        means = consts.tile([P, NT], dt.float32)
        sigs = consts.tile([P, NT], dt.float32)
        rsigs = consts.tile([P, NT], dt.float32)
        for i in range(NT):
            xt = xpool.tile([P, H], dt.float32)
            nc.sync.dma_start(xt[:], x[P * i : P * (i + 1), :])
            st = stat.tile([P, 2, 6], dt.float32)
            nc.vector.bn_stats(st[:, 0, :], xt[:, 0:512])
            nc.vector.bn_stats(st[:, 1, :], xt[:, 512:1024])
            mv = stat.tile([P, 2], dt.float32)
            nc.vector.bn_aggr(mv[:], st[:])
            nc.vector.tensor_copy(means[:, i : i + 1], mv[:, 0:1])
            nc.scalar.activation(
                out=sigs[:, i : i + 1], in_=mv[:, 1:2], func=AF.Sqrt,
                bias=eps[:], scale=1.0,
            )
        nc.vector.reciprocal(rsigs[:], sigs[:])

        # ---- phase 1: main loop ----
        for i in range(NT):
            xt = xpool.tile([P, H], dt.float32)
            nc.sync.dma_start(xt[:], x[P * i : P * (i + 1), :])
            xn = work.tile([P, H], dt.float16, tag="xn")
            nc.vector.tensor_scalar(
                out=xn[:], in0=xt[:],
                scalar1=means[:, i : i + 1], scalar2=rsigs[:, i : i + 1],
                op0=mybir.AluOpType.subtract, op1=mybir.AluOpType.mult,
            )
            # transpose xn -> xnT  [h-tile][128h, 128t]
            xnT = work.tile([P, NH, P], dt.float16, tag="xnT")
            for h in range(NH):
                ptt = pt.tile([P, P], dt.float16, tag="ptt")
                nc.tensor.transpose(ptt[:], xn[:, P * h : P * (h + 1)], ident[:])
                nc.vector.tensor_copy(xnT[:, h, :], ptt[:])

            # u projection  (psum [128,1])
            put = pu.tile([P, 1], dt.float32, tag="pu")
            for h in range(NH):
                nc.tensor.matmul(
                    put[:], xnT[:, h, :], w_sb[:, h, 0:1],
                    start=(h == 0), stop=(h == NH - 1),
                )
            u_col = work.tile([P, 1], dt.float32, tag="ucol")
            nc.scalar.copy(u_col[:], put[:])

            # 8 projection chunks of 512: [v0 v1 g0 g1 t0 t1 r0 r1]
            dact = work.tile([P, 2, 512], dt.float16, tag="dact")
            dgh = work.tile([P, 2, 512], dt.float16, tag="dgh")
            gate = work.tile([P, H], dt.float32, tag="gate")
            trans = work.tile([P, H], dt.float32, tag="trans")
            rt = work.tile([P, H], dt.float32, tag="rt")
            for j in range(8):
                ppt = pp.tile([P, 512], dt.float32, tag="ppt")
                c0 = 1 + 512 * j
                for h in range(NH):
                    nc.tensor.matmul(
                        ppt[:], xnT[:, h, :], w_sb[:, h, c0 : c0 + 512],
                        start=(h == 0), stop=(h == NH - 1),
                    )
                if j < 2:        # v -> delta_act = u * v
                    nc.vector.tensor_scalar_mul(dact[:, j, :], in0=ppt[:], scalar1=u_col[:])
                elif j < 4:      # g -> sigmoid
                    nc.scalar.activation(
                        out=gate[:, 512 * (j - 2) : 512 * (j - 1)], in_=ppt[:],
                        func=AF.Sigmoid, bias=0.0, scale=1.0,
                    )
                elif j < 6:      # t -> tanh, then delta_gh = gate * tanh (f16)
                    cj = j - 4
                    nc.scalar.activation(
                        out=trans[:, 512 * cj : 512 * (cj + 1)], in_=ppt[:],
                        func=AF.Tanh, bias=0.0, scale=1.0,
                    )
                    nc.vector.tensor_mul(
                        dgh[:, cj, :],
                        gate[:, 512 * cj : 512 * (cj + 1)],
                        trans[:, 512 * cj : 512 * (cj + 1)],
                    )
                else:            # r -> sigmoid
                    cj = j - 6
                    nc.scalar.activation(
                        out=rt[:, 512 * cj : 512 * (cj + 1)], in_=ppt[:],
                        func=AF.Sigmoid, bias=0.0, scale=1.0,
                    )

            # blocked cumsum + carry, then c = rt * (active + ghost)
            c_h = work.tile([P, H], dt.float16, tag="ch")
            for c in range(2):
                sl = slice(512 * c, 512 * (c + 1))
                pa = pcs.tile([P, 512], dt.float32, tag="pcs")
                nc.tensor.matmul(pa[:], umask[:], dact[:, c, :], start=True, stop=False)
                nc.tensor.matmul(pa[:], ones_col[:], cah[0:1, sl], start=False, stop=True)
                pg = pcs.tile([P, 512], dt.float32, tag="pcs")
                nc.tensor.matmul(pg[:], umask[:], dgh[:, c, :], start=True, stop=False)
                nc.tensor.matmul(pg[:], ones_col[:], cgh[0:1, sl], start=False, stop=True)

                # evacuate: tmpa = pa ; comb = pa + pg ; c = comb * rt
                tmpa = work.tile([P, 512], dt.float32, tag="tmpa")
                nc.scalar.copy(tmpa[:], pa[:])
                comb = work.tile([P, 512], dt.float32, tag="comb")
                nc.vector.tensor_add(comb[:], tmpa[:], pg[:])
                nc.vector.tensor_mul(c_h[:, sl], comb[:], rt[:, sl])

                # carry master update: block column-sums via ones-column matmul
                # (lands at psum partition 0 -- row-127 slices are not legal)
                pta = ptot.tile([1, 512], dt.float32, tag="ptot")
                nc.tensor.matmul(pta[:], ones128[:], dact[:, c, :], start=True, stop=True)
                nc.vector.tensor_add(ca[0:1, sl], ca[0:1, sl], pta[0:1, :])
                nc.vector.tensor_copy(cah[0:1, sl], ca[0:1, sl])
                ptg = ptot.tile([1, 512], dt.float32, tag="ptot")
                nc.tensor.matmul(ptg[:], ones128[:], dgh[:, c, :], start=True, stop=True)
                nc.vector.tensor_add(cg[0:1, sl], cg[0:1, sl], ptg[0:1, :])
                nc.vector.tensor_copy(cgh[0:1, sl], cg[0:1, sl])

            # transpose c -> cT, then out tile = cT.T @ WoT
            cT = work.tile([P, NH, P], dt.float16, tag="cT")
            for h in range(NH):
                ptt = pt.tile([P, P], dt.float16, tag="ptt")
                nc.tensor.transpose(ptt[:], c_h[:, P * h : P * (h + 1)], ident[:])
                nc.vector.tensor_copy(cT[:, h, :], ptt[:])

            ost = work.tile([P, H], dt.float32, tag="ost")
            for oc in range(2):
                pw = pwo.tile([P, 512], dt.float32, tag="pwo")
                for h in range(NH):
                    nc.tensor.matmul(
                        pw[:], cT[:, h, :], wo_sb[:, h, 512 * oc : 512 * (oc + 1)],
                        start=(h == 0), stop=(h == NH - 1),
                    )
                nc.scalar.copy(ost[:, 512 * oc : 512 * (oc + 1)], pw[:])
            nc.sync.dma_start(out[P * i : P * (i + 1), :], ost[:])

        # ---- final states ----
        nc.sync.dma_start(alast[:], ca[:])
        gl = consts.tile([1, H], dt.float16)
        nc.vector.tensor_copy(gl[:], cg[:])
        nc.sync.dma_start(glast[:], gl[:])

    nc.compile()
    return nc


def _get_nc():
    if "nc" not in _CACHE:
        _CACHE["nc"] = _build()
    return _CACHE["nc"]


def kernel(x, ln_gamma, ln_beta, Wp, bp, Wg, Wt, Wr, Wo):
    from concourse.bass_utils import run_bass_kernel_spmd

    x = np.asarray(x, dtype=np.float32)
    g = np.asarray(ln_gamma, dtype=np.float32)
    Wp = np.asarray(Wp, dtype=np.float32)
    Wg = np.asarray(Wg, dtype=np.float32)
    Wt = np.asarray(Wt, dtype=np.float32)
    Wr = np.asarray(Wr, dtype=np.float32)
    Wo = np.asarray(Wo, dtype=np.float32)

    wcat = np.empty((H, OC), dtype=np.float16)
    wcat[:, 0] = (Wp[0] * g).astype(np.float16)
    wcat[:, 1:1025] = (Wp[1:] * g).T.astype(np.float16)
    wcat[:, 1025:2049] = (Wg * g).T.astype(np.float16)
    wcat[:, 2049:3073] = (Wt * g).T.astype(np.float16)
    wcat[:, 3073:4097] = (Wr * g).T.astype(np.float16)
    wot = np.ascontiguousarray(Wo.T).astype(np.float16)

    nc = _get_nc()
    in_maps = [
        {"x": np.ascontiguousarray(x[b]), "wcat": wcat, "wot": wot}
        for b in range(B)
    ]
    res = run_bass_kernel_spmd(nc, in_maps, list(range(N_CORES)))

    out = np.stack([res.results[b]["out"] for b in range(B)])
    al = np.stack([res.results[b]["alast"] for b in range(B)])
    gl = np.stack([res.results[b]["glast"] for b in range(B)])
    return out, al, gl
